# revision 12
# baseline (speedup 1.0000x reference)
"""Contrastive queue loss kernel for 8 Trainium2 NeuronCores.

Reference computation (all fp32):
    pos[j,b]    = V[j,b,:] . L[b,:] / T                  (J=2, B=256, F=128)
    qlog[j,b,q] = V[j,b,:] . queue[q,:] / T              (Q=65536)
    denom[j,b]  = log( sum_i exp(pos[j,i]) + sum_q exp(qlog[j,b,q]) )
    loss        = -sum_{j,b} (pos[j,b] - denom[j,b]) / B

Sharding: queue split along Q across 8 cores (8192 rows each); V/L replicated.
Each core emits pos[jb] (fp32 dot products) and its partial
sum_q exp(10*logit) per jb; logits come from bf16 matmuls (fp32 PSUM
accumulate). Inputs are L2-normalized so |logit| <= 1 and
exp(10*logit) <= e^10 — no max-subtraction needed for fp32 safety.
Host combines partials with a tiny (4096-element) logsumexp in float64.

Per-core dataflow (TRANSPOSE_MODE == "dma"):
  DMA queue chunk (1024 rows, fp32) -> DVE cast to bf16
  -> one xbar DMA transpose per chunk -> queueT slices (bf16, SBUF)
  -> PE matmul against persistent V2T (bf16) -> logits in PSUM (fp32)
  -> ACT exp(10x) with fused free-dim accumulation (accum_out, FD=2048)
  -> DVE reduces partial columns, DMA out [2,128,4].
"pe" mode replaces the xbar transposes with PE identity-matmul
transposes through PSUM (FD=1536 ACT groups; 2 PSUM banks go to the
transposes) and pre-warms the PE clock with dummy transposes.
"""

import numpy as np

J, B, F, Q = 2, 256, 128, 65536
NCORES = 8
QC = Q // NCORES          # 8192 queue rows per core
JB = J * B                # 512
INV_T = 10.0
NT = JB // 128            # 4 jb tiles of 128
CHUNK = 1024              # queue rows per chunk (8 blocks of 128)
NCHUNK = QC // CHUNK      # 8
NSLICE = QC // 512        # 16 rhs slices of 512 q rows

TRANSPOSE_MODE = "dma"    # "dma" (xbar) or "pe" (identity matmul)
N_WARMUP = 40             # dummy PE ops to lift the HAM clock gate ("pe" mode)

_STATE = {}


def _groups():
    """(start_slice, n_slices) extents of the fused exp+accumulate ops."""
    if TRANSPOSE_MODE == "dma":
        # FD=2048: PSUM = 2 x 4 banks, uniform groups
        return [(s, 4) for s in range(0, NSLICE, 4)]
    # FD=1536: PSUM = 2 x 3 banks (logits) + 2 x 1 bank (transposes)
    return [(0, 3), (3, 3), (6, 3), (9, 3), (12, 3), (15, 1)]


def _build():
    import concourse.tile as tile
    from concourse import bacc, masks, mybir

    f32 = mybir.dt.float32
    bf16 = mybir.dt.bfloat16
    nc = bacc.Bacc("TRN2", target_bir_lowering=False, debug=False,
                   num_devices=NCORES)

    v2_d = nc.dram_tensor("V2", (JB, F), f32, kind="ExternalInput")
    l_d = nc.dram_tensor("L", (B, F), f32, kind="ExternalInput")
    q_d = nc.dram_tensor("queue", (QC, F), f32, kind="ExternalInput")
    # out[0, p, t] = pos_raw[jb = t*128 + p]
    # out[1, p, t] = sum over this core's queue shard of exp(10 * logit[jb, q])
    out_d = nc.dram_tensor("out", (2, 128, NT), f32, kind="ExternalOutput")

    groups = _groups()
    ng = len(groups)
    use_dma_t = TRANSPOSE_MODE == "dma"

    with tile.TileContext(nc) as tc:
        with (
            tc.tile_pool(name="const", bufs=1) as const_pool,
            tc.tile_pool(name="vl", bufs=1) as vl_pool,
            tc.tile_pool(name="qt", bufs=3) as qt_pool,
            tc.tile_pool(name="qtb", bufs=3) as qtb_pool,
            tc.tile_pool(name="qts", bufs=6) as qts_pool,
            tc.tile_pool(name="trash", bufs=2) as trash_pool,
            tc.tile_pool(name="res", bufs=1) as res_pool,
            tc.tile_pool(name="pslog", bufs=2, space="PSUM") as pslog_pool,
        ):
            if not use_dma_t:
                pst_pool = tc.tile_pool(name="pst", bufs=2, space="PSUM")
                pst_pool.__enter__()
                # PE clock warmup: garbage transposes on a zeroed tile,
                # emitted first so they run while input DMAs are in flight.
                wsrc = const_pool.tile([128, 128], bf16, tag="wsrc")
                nc.vector.memset(wsrc[:], 0.0)
                wdst = pst_pool.tile([128, CHUNK], bf16, tag="pst")
                for i in range(N_WARMUP):
                    nc.tensor.transpose(
                        wdst[:, (i % 8) * 128:(i % 8 + 1) * 128], wsrc[:],
                        wsrc[:])

            identb = const_pool.tile([128, 128], bf16)
            masks.make_identity(nc, identb[:])

            # ---- setup: V2T [f=128, jb=512] bf16 + pos[jb] fp32 ----
            vt_all = vl_pool.tile([128, JB], f32)      # [p, (t f)] natural V2
            nc.sync.dma_start(
                vt_all[:].rearrange("p (t f) -> p t f", f=F),
                v2_d.ap().rearrange("(t p) f -> p t f", p=128))
            lt = vl_pool.tile([128, B], f32)           # [p, (u f)] natural L
            nc.sync.dma_start(
                lt[:].rearrange("p (u f) -> p u f", f=F),
                l_d.ap().rearrange("(u p) f -> p u f", p=128))

            vtb = vl_pool.tile([128, JB], bf16)
            nc.vector.tensor_copy(vtb[:], vt_all[:])
            v2tb = vl_pool.tile([128, JB], bf16)       # [f, jb]
            if use_dma_t:
                nc.sync.dma_start_transpose(
                    v2tb[:].rearrange("p (t q) -> p t q", q=128), vtb[:])
            else:
                pv = pst_pool.tile([128, CHUNK], bf16, tag="pst")
                for t in range(NT):
                    nc.tensor.transpose(
                        pv[:, t * 128:(t + 1) * 128],
                        vtb[:, t * 128:(t + 1) * 128], identb[:])
                nc.vector.tensor_copy(v2tb[:], pv[:, :JB])

            pos_sb = res_pool.tile([128, NT], f32)
            junk = trash_pool.tile([128, 128], f32, tag="junk")
            for t in range(NT):
                u = t % (B // 128)
                nc.vector.tensor_mul(
                    junk[:],
                    vt_all[:, t * 128:(t + 1) * 128],
                    lt[:, u * 128:(u + 1) * 128])
                nc.vector.tensor_reduce(
                    out=pos_sb[:, t:t + 1], in_=junk[:],
                    axis=mybir.AxisListType.X, op=mybir.AluOpType.add)

            # ---- stream queue chunks: load, cast, transpose ----
            qv = q_d.ap().rearrange("(c s p) f -> c p s f", p=128, s=CHUNK // 128)
            slices = []                                # 16 x [f=128, q=512] bf16
            for c in range(NCHUNK):
                qt = qt_pool.tile([128, CHUNK], f32)
                nc.sync.dma_start(
                    qt[:].rearrange("p (s f) -> p s f", f=F), qv[c])
                qtb = qtb_pool.tile([128, CHUNK], bf16)
                nc.vector.tensor_copy(qtb[:], qt[:])
                qts = qts_pool.tile([128, CHUNK], bf16)
                if use_dma_t:
                    nc.sync.dma_start_transpose(
                        qts[:].rearrange("p (s q) -> p s q", q=128), qtb[:])
                else:
                    pt = pst_pool.tile([128, CHUNK], bf16, tag="pst")
                    for s in range(CHUNK // 128):
                        nc.tensor.transpose(
                            pt[:, s * 128:(s + 1) * 128],
                            qtb[:, s * 128:(s + 1) * 128], identb[:])
                    nc.vector.tensor_copy(qts[:], pt[:])
                slices.append(qts[:, 0:512])
                slices.append(qts[:, 512:CHUNK])

            # ---- logits + fused exp/accumulate ----
            # acc[p, t*ng + g] = partial sum for jb tile t, ACT group g
            acc = res_pool.tile([128, NT * ng], f32)
            for gi, (s0, ns) in enumerate(groups):
                for t in range(NT):
                    lg = pslog_pool.tile([128, 512 * ns], f32, tag="pslog")
                    for k in range(ns):
                        nc.tensor.matmul(
                            lg[:, k * 512:(k + 1) * 512],
                            lhsT=v2tb[:, t * 128:(t + 1) * 128],
                            rhs=slices[s0 + k], start=True, stop=True)
                    tr = trash_pool.tile([128, 512 * ns], f32, tag="tr")
                    col = t * ng + gi
                    nc.scalar.activation(
                        tr[:], lg[:], mybir.ActivationFunctionType.Exp,
                        scale=INV_T, accum_out=acc[:, col:col + 1])

            # ---- finalize: reduce partials over groups, DMA out ----
            s_sb = res_pool.tile([128, NT], f32)
            for t in range(NT):
                nc.vector.tensor_reduce(
                    out=s_sb[:, t:t + 1],
                    in_=acc[:, t * ng:(t + 1) * ng],
                    axis=mybir.AxisListType.X, op=mybir.AluOpType.add)
            nc.sync.dma_start(out_d.ap()[0], pos_sb[:])
            nc.sync.dma_start(out_d.ap()[1], s_sb[:])

    nc.compile()
    return nc


def _run(in_maps, trace=False, **kwargs):
    from concourse.bass_utils import run_bass_kernel_spmd
    if "nc" not in _STATE:
        _STATE["nc"] = _build()
    return run_bass_kernel_spmd(_STATE["nc"], in_maps, list(range(NCORES)),
                                trace=trace, **kwargs)


def _make_in_maps(V, L, queue):
    V2 = np.ascontiguousarray(
        np.asarray(V, dtype=np.float32).reshape(JB, F))
    Ln = np.ascontiguousarray(np.asarray(L, dtype=np.float32))
    qn = np.asarray(queue, dtype=np.float32).reshape(NCORES, QC, F)
    return [{"V2": V2, "L": Ln, "queue": np.ascontiguousarray(qn[i])}
            for i in range(NCORES)]


def _combine(outs):
    """outs: list of (2, 128, NT) arrays, one per core -> scalar loss."""
    pos_raw = outs[0][0].T.reshape(JB).astype(np.float64)   # jb = t*128 + p
    qsum = np.zeros(JB, dtype=np.float64)
    for o in outs:
        qsum += o[1].T.reshape(JB).astype(np.float64)
    pos_s = INV_T * pos_raw
    batch_sum = np.exp(pos_s).reshape(J, B).sum(axis=1)     # sum_i exp(pos[j,i])
    denom = np.log(np.repeat(batch_sum, B) + qsum)
    loss = -(pos_s.sum() - denom.sum()) / B
    return np.array(loss, dtype=np.float32)


def kernel(V, L, queue):
    res = _run(_make_in_maps(V, L, queue))
    return _combine([res.results[i]["out"] for i in range(NCORES)])


# revision 15
# speedup vs baseline: 1.3579x; 1.3579x over previous
"""Contrastive queue loss kernel for 8 Trainium2 NeuronCores.

Reference computation (all fp32):
    pos[j,b]    = V[j,b,:] . L[b,:] / T                  (J=2, B=256, F=128)
    qlog[j,b,q] = V[j,b,:] . queue[q,:] / T              (Q=65536)
    denom[j,b]  = log( sum_i exp(pos[j,i]) + sum_q exp(qlog[j,b,q]) )
    loss        = -sum_{j,b} (pos[j,b] - denom[j,b]) / B

Sharding: queue split along Q across 8 cores (8192 rows each); V/L replicated.
Each core emits pos[jb] (fp32 dot products) and its partial
sum_q exp(10*logit) per jb; logits come from bf16 matmuls (fp32 PSUM
accumulate). Inputs are L2-normalized so |logit| <= 1 and
exp(10*logit) <= e^10 — no max-subtraction needed for fp32 safety.
Host combines partials with a tiny (4096-element) logsumexp in float64.

Per-core dataflow (TRANSPOSE_MODE == "dma"):
  DMA queue chunk (1024 rows, fp32) -> DVE cast to bf16
  -> one xbar DMA transpose per chunk -> queueT slices (bf16, SBUF)
  -> PE matmul against persistent V2T (bf16) -> logits in PSUM (fp32)
  -> ACT exp(10x) with fused free-dim accumulation (accum_out, FD=2048)
  -> DVE reduces partial columns, DMA out [2,128,4].
"pe" mode replaces the xbar transposes with PE identity-matmul
transposes through PSUM (FD=1536 ACT groups; 2 PSUM banks go to the
transposes) and pre-warms the PE clock with dummy transposes.
"""

import numpy as np

J, B, F, Q = 2, 256, 128, 65536
NCORES = 8
QC = Q // NCORES          # 8192 queue rows per core
JB = J * B                # 512
INV_T = 10.0
NT = JB // 128            # 4 jb tiles of 128
CHUNK = 1024              # queue rows per chunk (8 blocks of 128)
NCHUNK = QC // CHUNK      # 8
NSLICE = QC // 512        # 16 rhs slices of 512 q rows

TRANSPOSE_MODE = "pe"    # "dma" (xbar) or "pe" (identity matmul)
N_WARMUP = 40             # dummy PE ops to lift the HAM clock gate ("pe" mode)

_STATE = {}


def _groups():
    """(start_slice, n_slices) extents of the fused exp+accumulate ops."""
    if TRANSPOSE_MODE == "dma":
        # FD=2048: PSUM = 2 x 4 banks, uniform groups
        return [(s, 4) for s in range(0, NSLICE, 4)]
    # FD=1536: PSUM = 2 x 3 banks (logits) + 2 x 1 bank (transposes)
    return [(0, 3), (3, 3), (6, 3), (9, 3), (12, 3), (15, 1)]


def _build():
    import concourse.tile as tile
    from concourse import bacc, masks, mybir

    f32 = mybir.dt.float32
    bf16 = mybir.dt.bfloat16
    nc = bacc.Bacc("TRN2", target_bir_lowering=False, debug=False,
                   num_devices=NCORES)

    v2_d = nc.dram_tensor("V2", (JB, F), f32, kind="ExternalInput")
    l_d = nc.dram_tensor("L", (B, F), f32, kind="ExternalInput")
    q_d = nc.dram_tensor("queue", (QC, F), f32, kind="ExternalInput")
    # out[0, p, t] = pos_raw[jb = t*128 + p]
    # out[1, p, t] = sum over this core's queue shard of exp(10 * logit[jb, q])
    out_d = nc.dram_tensor("out", (2, 128, NT), f32, kind="ExternalOutput")

    groups = _groups()
    ng = len(groups)
    use_dma_t = TRANSPOSE_MODE == "dma"

    with tile.TileContext(nc) as tc:
        with (
            tc.tile_pool(name="const", bufs=1) as const_pool,
            tc.tile_pool(name="vl", bufs=1) as vl_pool,
            tc.tile_pool(name="qt", bufs=3) as qt_pool,
            tc.tile_pool(name="qtb", bufs=3) as qtb_pool,
            tc.tile_pool(name="qts", bufs=6) as qts_pool,
            tc.tile_pool(name="trash", bufs=2) as trash_pool,
            tc.tile_pool(name="res", bufs=1) as res_pool,
            tc.tile_pool(name="pslog", bufs=2, space="PSUM") as pslog_pool,
            tc.tile_pool(name="pst", bufs=2, space="PSUM") as pst_pool,
        ):
            if not use_dma_t:
                # PE clock warmup: garbage transposes on a zeroed tile,
                # emitted first so they run while input DMAs are in flight.
                wsrc = const_pool.tile([128, 128], bf16, tag="wsrc")
                nc.vector.memset(wsrc[:], 0.0)
                wdst = pst_pool.tile([128, CHUNK], bf16, tag="pst")
                for i in range(N_WARMUP):
                    nc.tensor.transpose(
                        wdst[:, (i % 8) * 128:(i % 8 + 1) * 128], wsrc[:],
                        wsrc[:])

            identb = const_pool.tile([128, 128], bf16)
            masks.make_identity(nc, identb[:])

            # ---- setup: V2T [f=128, jb=512] bf16 + pos[jb] fp32 ----
            vt_all = vl_pool.tile([128, JB], f32)      # [p, (t f)] natural V2
            nc.sync.dma_start(
                vt_all[:].rearrange("p (t f) -> p t f", f=F),
                v2_d.ap().rearrange("(t p) f -> p t f", p=128))
            lt = vl_pool.tile([128, B], f32)           # [p, (u f)] natural L
            nc.sync.dma_start(
                lt[:].rearrange("p (u f) -> p u f", f=F),
                l_d.ap().rearrange("(u p) f -> p u f", p=128))

            vtb = vl_pool.tile([128, JB], bf16)
            nc.vector.tensor_copy(vtb[:], vt_all[:])
            v2tb = vl_pool.tile([128, JB], bf16)       # [f, jb]
            if use_dma_t:
                nc.sync.dma_start_transpose(
                    v2tb[:].rearrange("p (t q) -> p t q", q=128), vtb[:])
            else:
                pv = pst_pool.tile([128, CHUNK], bf16, tag="pst")
                for t in range(NT):
                    nc.tensor.transpose(
                        pv[:, t * 128:(t + 1) * 128],
                        vtb[:, t * 128:(t + 1) * 128], identb[:])
                nc.vector.tensor_copy(v2tb[:], pv[:, :JB])

            pos_sb = res_pool.tile([128, NT], f32)
            junk = trash_pool.tile([128, 128], f32, tag="junk")
            for t in range(NT):
                u = t % (B // 128)
                nc.vector.tensor_mul(
                    junk[:],
                    vt_all[:, t * 128:(t + 1) * 128],
                    lt[:, u * 128:(u + 1) * 128])
                nc.vector.tensor_reduce(
                    out=pos_sb[:, t:t + 1], in_=junk[:],
                    axis=mybir.AxisListType.X, op=mybir.AluOpType.add)

            # ---- stream queue chunks: load, cast, transpose ----
            qv = q_d.ap().rearrange("(c s p) f -> c p s f", p=128, s=CHUNK // 128)
            slices = []                                # 16 x [f=128, q=512] bf16
            for c in range(NCHUNK):
                qt = qt_pool.tile([128, CHUNK], f32)
                nc.sync.dma_start(
                    qt[:].rearrange("p (s f) -> p s f", f=F), qv[c])
                qtb = qtb_pool.tile([128, CHUNK], bf16)
                nc.vector.tensor_copy(qtb[:], qt[:])
                qts = qts_pool.tile([128, CHUNK], bf16)
                if use_dma_t:
                    nc.sync.dma_start_transpose(
                        qts[:].rearrange("p (s q) -> p s q", q=128), qtb[:])
                else:
                    pt = pst_pool.tile([128, CHUNK], bf16, tag="pst")
                    for s in range(CHUNK // 128):
                        nc.tensor.transpose(
                            pt[:, s * 128:(s + 1) * 128],
                            qtb[:, s * 128:(s + 1) * 128], identb[:])
                    nc.vector.tensor_copy(qts[:], pt[:])
                slices.append(qts[:, 0:512])
                slices.append(qts[:, 512:CHUNK])

            # ---- logits + fused exp/accumulate ----
            # acc[p, t*ng + g] = partial sum for jb tile t, ACT group g
            acc = res_pool.tile([128, NT * ng], f32)
            for gi, (s0, ns) in enumerate(groups):
                for t in range(NT):
                    lg = pslog_pool.tile([128, 512 * ns], f32, tag="pslog")
                    for k in range(ns):
                        nc.tensor.matmul(
                            lg[:, k * 512:(k + 1) * 512],
                            lhsT=v2tb[:, t * 128:(t + 1) * 128],
                            rhs=slices[s0 + k], start=True, stop=True)
                    tr = trash_pool.tile([128, 512 * ns], f32, tag="tr")
                    col = t * ng + gi
                    nc.scalar.activation(
                        tr[:], lg[:], mybir.ActivationFunctionType.Exp,
                        scale=INV_T, accum_out=acc[:, col:col + 1])

            # ---- finalize: reduce partials over groups, DMA out ----
            s_sb = res_pool.tile([128, NT], f32)
            for t in range(NT):
                nc.vector.tensor_reduce(
                    out=s_sb[:, t:t + 1],
                    in_=acc[:, t * ng:(t + 1) * ng],
                    axis=mybir.AxisListType.X, op=mybir.AluOpType.add)
            nc.sync.dma_start(out_d.ap()[0], pos_sb[:])
            nc.sync.dma_start(out_d.ap()[1], s_sb[:])

    nc.compile()
    return nc


def _run(in_maps, trace=False, **kwargs):
    from concourse.bass_utils import run_bass_kernel_spmd
    if "nc" not in _STATE:
        _STATE["nc"] = _build()
    return run_bass_kernel_spmd(_STATE["nc"], in_maps, list(range(NCORES)),
                                trace=trace, **kwargs)


def _make_in_maps(V, L, queue):
    V2 = np.ascontiguousarray(
        np.asarray(V, dtype=np.float32).reshape(JB, F))
    Ln = np.ascontiguousarray(np.asarray(L, dtype=np.float32))
    qn = np.asarray(queue, dtype=np.float32).reshape(NCORES, QC, F)
    return [{"V2": V2, "L": Ln, "queue": np.ascontiguousarray(qn[i])}
            for i in range(NCORES)]


def _combine(outs):
    """outs: list of (2, 128, NT) arrays, one per core -> scalar loss."""
    pos_raw = outs[0][0].T.reshape(JB).astype(np.float64)   # jb = t*128 + p
    qsum = np.zeros(JB, dtype=np.float64)
    for o in outs:
        qsum += o[1].T.reshape(JB).astype(np.float64)
    pos_s = INV_T * pos_raw
    batch_sum = np.exp(pos_s).reshape(J, B).sum(axis=1)     # sum_i exp(pos[j,i])
    denom = np.log(np.repeat(batch_sum, B) + qsum)
    loss = -(pos_s.sum() - denom.sum()) / B
    return np.array(loss, dtype=np.float32)


def kernel(V, L, queue):
    res = _run(_make_in_maps(V, L, queue))
    return _combine([res.results[i]["out"] for i in range(NCORES)])


# revision 17
# speedup vs baseline: 1.4608x; 1.0758x over previous
"""Contrastive queue loss kernel for 8 Trainium2 NeuronCores.

Reference computation (all fp32):
    pos[j,b]    = V[j,b,:] . L[b,:] / T                  (J=2, B=256, F=128)
    qlog[j,b,q] = V[j,b,:] . queue[q,:] / T              (Q=65536)
    denom[j,b]  = log( sum_i exp(pos[j,i]) + sum_q exp(qlog[j,b,q]) )
    loss        = -sum_{j,b} (pos[j,b] - denom[j,b]) / B

Sharding: queue split along Q across 8 cores (8192 rows each); V/L replicated.
Each core emits pos[jb] (fp32 dot products) and its partial
sum_q exp(10*logit) per jb; logits come from bf16 matmuls (fp32 PSUM
accumulate). Inputs are L2-normalized so |logit| <= 1 and
exp(10*logit) <= e^10 — no max-subtraction needed for fp32 safety.
Host combines partials with a tiny (4096-element) logsumexp in float64.

Per-core dataflow:
  DMA queue chunk (fp32) -> DVE cast to bf16
  -> PE 128x128 identity-matmul transposes into PSUM -> DVE copy to SBUF
  -> PE matmul against persistent V2T (bf16) -> logits in PSUM (fp32)
  -> ACT exp(10x) in place, with fused free-dim accumulation (accum_out)
  -> DVE reduces partial columns, DMA out [2,128,4].
Ramp tricks: dummy LDWEIGHTS spam lifts the PE HAM clock gate while the
first DMAs are in flight; the first two chunks are 512 rows and the
first ACT group per jb tile is a single 512-slice so the Scalar engine
starts as early as possible.
"""

import numpy as np

J, B, F, Q = 2, 256, 128, 65536
NCORES = 8
QC = Q // NCORES          # 8192 queue rows per core
JB = J * B                # 512
INV_T = 10.0
NT = JB // 128            # 4 jb tiles of 128
NSLICE = QC // 512        # 16 rhs slices of 512 q rows

# (row_start, nrows) queue chunks: small first chunks shorten the ramp.
CHUNKS = [(0, 512), (512, 512)] + [(r, 1024) for r in range(1024, QC, 1024)]
# (start_slice, n_slices) extents of the fused exp+accumulate ACT ops.
# PSUM: 2 x 3 banks (logits, FD<=1536) + 2 x 1 bank (transposes) = 8.
GROUPS = [(0, 1), (1, 3), (4, 3), (7, 3), (10, 3), (13, 3)]
NG = len(GROUPS)
N_WARMUP = 80             # dummy PE weight loads to lift the HAM clock gate

_STATE = {}


def _build():
    import concourse.tile as tile
    from concourse import bacc, masks, mybir

    f32 = mybir.dt.float32
    bf16 = mybir.dt.bfloat16
    nc = bacc.Bacc("TRN2", target_bir_lowering=False, debug=False,
                   num_devices=NCORES)

    v2_d = nc.dram_tensor("V2", (JB, F), f32, kind="ExternalInput")
    l_d = nc.dram_tensor("L", (B, F), f32, kind="ExternalInput")
    q_d = nc.dram_tensor("queue", (QC, F), f32, kind="ExternalInput")
    # out[0, p, t] = pos_raw[jb = t*128 + p]
    # out[1, p, t] = sum over this core's queue shard of exp(10 * logit[jb, q])
    out_d = nc.dram_tensor("out", (2, 128, NT), f32, kind="ExternalOutput")

    with tile.TileContext(nc) as tc:
        with (
            tc.tile_pool(name="const", bufs=1) as const_pool,
            tc.tile_pool(name="vl", bufs=1) as vl_pool,
            tc.tile_pool(name="qt", bufs=3) as qt_pool,
            tc.tile_pool(name="qtb", bufs=3) as qtb_pool,
            tc.tile_pool(name="qts", bufs=6) as qts_pool,
            tc.tile_pool(name="trash", bufs=2) as trash_pool,
            tc.tile_pool(name="res", bufs=1) as res_pool,
            tc.tile_pool(name="pslog", bufs=2, space="PSUM") as pslog_pool,
            tc.tile_pool(name="pst", bufs=2, space="PSUM") as pst_pool,
        ):
            # PE clock warmup: dependency-free weight loads keep the PE
            # busy from the end of its preamble so the HAM gate opens
            # before the first real transpose arrives.
            wsrc = const_pool.tile([128, 128], bf16, tag="wsrc")
            nc.vector.memset(wsrc[:], 0.0)
            for _ in range(N_WARMUP):
                nc.tensor.ldweights(wsrc[:])

            identb = const_pool.tile([128, 128], bf16)
            masks.make_identity(nc, identb[:])

            # V2T [f=128, jb=512] bf16 (needed by the first matmul group)
            vt_all = vl_pool.tile([128, JB], f32)      # [p, (t f)] natural V2
            nc.sync.dma_start(
                vt_all[:].rearrange("p (t f) -> p t f", f=F),
                v2_d.ap().rearrange("(t p) f -> p t f", p=128))
            vtb = vl_pool.tile([128, JB], bf16)
            nc.vector.tensor_copy(vtb[:], vt_all[:])
            pv = pst_pool.tile([128, 1024], bf16, tag="pst")
            for t in range(NT):
                nc.tensor.transpose(
                    pv[:, t * 128:(t + 1) * 128],
                    vtb[:, t * 128:(t + 1) * 128], identb[:])
            v2tb = vl_pool.tile([128, JB], bf16)       # [f, jb]
            nc.vector.tensor_copy(v2tb[:], pv[:, :JB])

            # ---- stream queue chunks: load, cast, transpose ----
            slices = []                                # 16 x [f=128, q=512] bf16
            for r0, nr in CHUNKS:
                nb = nr // 128
                qt = qt_pool.tile([128, nr], f32, tag="qt")
                nc.sync.dma_start(
                    qt[:].rearrange("p (s f) -> p s f", f=F),
                    q_d.ap()[r0:r0 + nr, :].rearrange(
                        "(s p) f -> p s f", p=128))
                qtb = qtb_pool.tile([128, nr], bf16, tag="qtb")
                nc.vector.tensor_copy(qtb[:], qt[:])
                pt = pst_pool.tile([128, nr], bf16, tag="pst")
                for s in range(nb):
                    nc.tensor.transpose(
                        pt[:, s * 128:(s + 1) * 128],
                        qtb[:, s * 128:(s + 1) * 128], identb[:])
                qts = qts_pool.tile([128, nr], bf16, tag="qts")
                nc.vector.tensor_copy(qts[:], pt[:])
                for s in range(nr // 512):
                    slices.append(qts[:, s * 512:(s + 1) * 512])

            # ---- logits + fused exp/accumulate (in-place on PSUM) ----
            # acc[p, t*NG + g] = partial sum for jb tile t, ACT group g
            acc = res_pool.tile([128, NT * NG], f32)
            for gi, (s0, ns) in enumerate(GROUPS):
                for t in range(NT):
                    lg = pslog_pool.tile([128, 512 * ns], f32, tag="pslog")
                    for k in range(ns):
                        nc.tensor.matmul(
                            lg[:, k * 512:(k + 1) * 512],
                            lhsT=v2tb[:, t * 128:(t + 1) * 128],
                            rhs=slices[s0 + k], start=True, stop=True)
                    col = t * NG + gi
                    nc.scalar.activation(
                        lg[:], lg[:], mybir.ActivationFunctionType.Exp,
                        scale=INV_T, accum_out=acc[:, col:col + 1])

            # ---- pos[jb] (fp32, off the critical path) ----
            lt = vl_pool.tile([128, B], f32)           # [p, (u f)] natural L
            nc.sync.dma_start(
                lt[:].rearrange("p (u f) -> p u f", f=F),
                l_d.ap().rearrange("(u p) f -> p u f", p=128))
            pos_sb = res_pool.tile([128, NT], f32)
            junk = trash_pool.tile([128, 128], f32, tag="junk")
            for t in range(NT):
                u = t % (B // 128)
                nc.vector.tensor_mul(
                    junk[:],
                    vt_all[:, t * 128:(t + 1) * 128],
                    lt[:, u * 128:(u + 1) * 128])
                nc.vector.tensor_reduce(
                    out=pos_sb[:, t:t + 1], in_=junk[:],
                    axis=mybir.AxisListType.X, op=mybir.AluOpType.add)

            # ---- finalize: reduce partials over groups, DMA out ----
            s_sb = res_pool.tile([128, NT], f32)
            for t in range(NT):
                nc.vector.tensor_reduce(
                    out=s_sb[:, t:t + 1],
                    in_=acc[:, t * NG:(t + 1) * NG],
                    axis=mybir.AxisListType.X, op=mybir.AluOpType.add)
            nc.sync.dma_start(out_d.ap()[0], pos_sb[:])
            nc.sync.dma_start(out_d.ap()[1], s_sb[:])

    nc.compile()
    return nc


def _run(in_maps, trace=False, **kwargs):
    from concourse.bass_utils import run_bass_kernel_spmd
    if "nc" not in _STATE:
        _STATE["nc"] = _build()
    return run_bass_kernel_spmd(_STATE["nc"], in_maps, list(range(NCORES)),
                                trace=trace, **kwargs)


def _make_in_maps(V, L, queue):
    V2 = np.ascontiguousarray(
        np.asarray(V, dtype=np.float32).reshape(JB, F))
    Ln = np.ascontiguousarray(np.asarray(L, dtype=np.float32))
    qn = np.asarray(queue, dtype=np.float32).reshape(NCORES, QC, F)
    return [{"V2": V2, "L": Ln, "queue": np.ascontiguousarray(qn[i])}
            for i in range(NCORES)]


def _combine(outs):
    """outs: list of (2, 128, NT) arrays, one per core -> scalar loss."""
    pos_raw = outs[0][0].T.reshape(JB).astype(np.float64)   # jb = t*128 + p
    qsum = np.zeros(JB, dtype=np.float64)
    for o in outs:
        qsum += o[1].T.reshape(JB).astype(np.float64)
    pos_s = INV_T * pos_raw
    batch_sum = np.exp(pos_s).reshape(J, B).sum(axis=1)     # sum_i exp(pos[j,i])
    denom = np.log(np.repeat(batch_sum, B) + qsum)
    loss = -(pos_s.sum() - denom.sum()) / B
    return np.array(loss, dtype=np.float32)


def kernel(V, L, queue):
    res = _run(_make_in_maps(V, L, queue))
    return _combine([res.results[i]["out"] for i in range(NCORES)])
